# revision 4
# baseline (speedup 1.0000x reference)
"""DeepPoly ReLU abstract-transformer kernel for 8 TRN2 NeuronCores.

Reference semantics (elementwise over N = 16,777,216):
    x_out     = relu(x)
    neg  = upper <= 0          -> bounds (0, 0)
    pos  = lower >= 0          -> bounds (upper, upper)
    crossing   (else)          -> (lower, upper^2 / (upper - lower))

Branch-free device formulation (all f32):
    up  = relu(upper)                 # ACT
    nl  = relu(-lower)                # ACT
    pp  = relu(lower)                 # ACT
    sq  = up^2                        # ACT (Square)
    d   = up + nl                     # GPSIMD   (== upper - lower on crossing)
    r   = 1/d                         # DVE reciprocal_approx_fast
    upper_out = sq * r                # DVE
      neg: 0/(-l) = 0; pos: u^2/u = u; crossing: u^2/(u-l)   -- exact cases fall out
    lower_out = 0                     # GPSIMD memset
    lower_out = where(up != 0, lower, lower_out)      # DVE copy_predicated
    lower_out = where(pp != 0, upper_out, lower_out)  # DVE copy_predicated

Sharding: pure elementwise -> split N across the 8 cores; each core sees a
[128, 16384] f32 view of its 2,097,152-element slice. No communication.
"""

import numpy as np

import concourse.bacc as bacc
import concourse.mybir as mybir
import concourse.tile as tile
from concourse import bass_utils

N_CORES = 8
N_TOTAL = 16777216
P = 128
NCOLS = N_TOTAL // N_CORES // P  # 16384
TILE_F = 2048

_F32 = mybir.dt.float32
_RELU = mybir.ActivationFunctionType.Relu
_SQUARE = mybir.ActivationFunctionType.Square


def build_nc(ncols: int = NCOLS, tile_f: int = TILE_F, bufs: int = 2):
    assert ncols % tile_f == 0
    nc = bacc.Bacc(
        "TRN2", target_bir_lowering=False, debug=False, num_devices=N_CORES
    )
    x = nc.dram_tensor("x", [P, ncols], _F32, kind="ExternalInput").ap()
    lo = nc.dram_tensor("lower", [P, ncols], _F32, kind="ExternalInput").ap()
    up = nc.dram_tensor("upper", [P, ncols], _F32, kind="ExternalInput").ap()
    xo = nc.dram_tensor("x_out", [P, ncols], _F32, kind="ExternalOutput").ap()
    loo = nc.dram_tensor("lower_out", [P, ncols], _F32, kind="ExternalOutput").ap()
    upo = nc.dram_tensor("upper_out", [P, ncols], _F32, kind="ExternalOutput").ap()

    with tile.TileContext(nc) as tc:
        with tc.tile_pool(name="io", bufs=bufs) as pool:
            for i in range(ncols // tile_f):
                sl = slice(i * tile_f, (i + 1) * tile_f)
                xt = pool.tile([P, tile_f], _F32, tag="x")
                lt = pool.tile([P, tile_f], _F32, tag="l")
                ut = pool.tile([P, tile_f], _F32, tag="u")
                nc.sync.dma_start(out=xt[:], in_=x[:, sl])
                nc.sync.dma_start(out=lt[:], in_=lo[:, sl])
                nc.sync.dma_start(out=ut[:], in_=up[:, sl])

                nc.scalar.activation(xt[:], xt[:], _RELU)  # x_out, in place
                nc.scalar.activation(ut[:], ut[:], _RELU)  # up = relu(u), in place
                nlt = pool.tile([P, tile_f], _F32, tag="nl")
                nc.scalar.activation(nlt[:], lt[:], _RELU, scale=-1.0)  # relu(-l)
                # exact masks; HW CopyPredicated requires an integer mask
                # dtype.  is_ge (not Relu(l)!) so l == 0.0 takes the pos
                # branch exactly like the reference; is_gt so u == -0.0
                # stays in the neg branch.
                ppt = pool.tile([P, tile_f], mybir.dt.uint8, tag="pp")
                nc.vector.tensor_scalar(
                    out=ppt[:], in0=lt[:], scalar1=0.0, scalar2=None,
                    op0=mybir.AluOpType.is_ge,
                )
                gt_ = pool.tile([P, tile_f], mybir.dt.uint8, tag="g")
                nc.gpsimd.tensor_scalar(
                    out=gt_[:], in0=ut[:], scalar1=0.0, scalar2=None,
                    op0=mybir.AluOpType.is_gt,
                )
                sqt = pool.tile([P, tile_f], _F32, tag="sq")
                nc.scalar.activation(sqt[:], ut[:], _SQUARE)  # up^2

                dt_ = pool.tile([P, tile_f], _F32, tag="d")
                nc.gpsimd.tensor_add(out=dt_[:], in0=ut[:], in1=nlt[:])
                lot = pool.tile([P, tile_f], _F32, tag="low")
                nc.gpsimd.memset(lot[:], 0.0)

                rt = pool.tile([P, tile_f], _F32, tag="r")
                nc.vector.reciprocal_approx_fast(out=rt[:], in_=dt_[:])
                uot = pool.tile([P, tile_f], _F32, tag="uo")
                nc.vector.tensor_mul(out=uot[:], in0=sqt[:], in1=rt[:])
                nc.vector.copy_predicated(out=lot[:], mask=gt_[:], data=lt[:])
                nc.vector.copy_predicated(out=lot[:], mask=ppt[:], data=uot[:])

                nc.sync.dma_start(out=xo[:, sl], in_=xt[:])
                nc.sync.dma_start(out=loo[:, sl], in_=lot[:])
                nc.sync.dma_start(out=upo[:, sl], in_=uot[:])
    nc.compile()
    return nc


def run(inputs: dict, trace: bool = False):
    """Shard, execute on 8 cores, gather. Returns (outputs_tuple, results_obj)."""
    arrs = {}
    for k in ("x", "lower", "upper"):
        a = np.asarray(inputs[k], dtype=np.float32)
        arrs[k] = np.ascontiguousarray(a).reshape(N_CORES, P, NCOLS)
    in_maps = [
        {k: arrs[k][c] for k in ("x", "lower", "upper")} for c in range(N_CORES)
    ]
    nc = build_nc()
    res = bass_utils.run_bass_kernel_spmd(
        nc, in_maps, core_ids=list(range(N_CORES)), trace=trace
    )
    outs = []
    for name in ("x_out", "lower_out", "upper_out"):
        full = np.stack([res.results[c][name] for c in range(N_CORES)])
        outs.append(full.reshape(1, N_TOTAL).astype(np.float32, copy=False))
    return tuple(outs), res


def kernel(**inputs):
    outs, _ = run(inputs, trace=False)
    return outs


# revision 8
# speedup vs baseline: 24.9647x; 24.9647x over previous
"""DeepPoly ReLU abstract-transformer kernel for 8 TRN2 NeuronCores.

Reference semantics (elementwise over N = 16,777,216):
    x_out     = relu(x)
    neg  = upper <= 0          -> bounds (0, 0)
    pos  = lower >= 0          -> bounds (upper, upper)
    crossing   (else)          -> (lower, upper^2 / (upper - lower))

Branch-free device formulation (all f32):
    up  = relu(upper)                      # ACT, in place on u
    nl  = relu(-lower)                     # ACT
    sq  = up^2                             # ACT (Square)
    pp  = (lower >= 0)  as uint8           # DVE  is_ge
    le  = (up <= 0)     as uint8           # GPSIMD is_le  (== upper <= 0)
    d   = up + nl                          # GPSIMD, in place on nl
    r   = 1/d                              # DVE reciprocal_approx_fast, in place
    upper_out = sq * r                     # DVE, in place on sq
      neg: 0*(1/-l) = 0; pos: u^2/u = u; crossing: u^2/(u-l)
    lower_out (in place on l):
      where(le) <- 0                       # DVE copy_predicated from zeros
      where(pp) <- upper_out               # DVE copy_predicated

Sharding: pure elementwise -> split N across the 8 cores; each core sees a
[128, 16384] f32 view of its 2,097,152-element slice. No communication.
"""

import numpy as np

import concourse.bacc as bacc
import concourse.mybir as mybir
import concourse.tile as tile
from concourse import bass_utils

N_CORES = 8
N_TOTAL = 16777216
P = 128
NCOLS = N_TOTAL // N_CORES // P  # 16384
TILE_F = 2048
BUFS = 3

_F32 = mybir.dt.float32
_U8 = mybir.dt.uint8
_RELU = mybir.ActivationFunctionType.Relu
_SQUARE = mybir.ActivationFunctionType.Square


def build_nc(
    ncols: int = NCOLS, tile_f: int = TILE_F, bufs: int = BUFS, reps: int = 1
):
    """reps > 1 repeats the whole pipeline in one NEFF (benchmarking only:
    lets wall-clock deltas cancel the per-launch dispatch overhead)."""
    assert ncols % tile_f == 0
    nc = bacc.Bacc(
        "TRN2", target_bir_lowering=False, debug=False, num_devices=N_CORES
    )
    x = nc.dram_tensor("x", [P, ncols], _F32, kind="ExternalInput").ap()
    lo = nc.dram_tensor("lower", [P, ncols], _F32, kind="ExternalInput").ap()
    up = nc.dram_tensor("upper", [P, ncols], _F32, kind="ExternalInput").ap()
    xo = nc.dram_tensor("x_out", [P, ncols], _F32, kind="ExternalOutput").ap()
    loo = nc.dram_tensor("lower_out", [P, ncols], _F32, kind="ExternalOutput").ap()
    upo = nc.dram_tensor("upper_out", [P, ncols], _F32, kind="ExternalOutput").ap()

    with tile.TileContext(nc) as tc:
        with (
            tc.tile_pool(name="const", bufs=1) as cpool,
            tc.tile_pool(name="io", bufs=bufs) as pool,
        ):
            zt = cpool.tile([P, tile_f], _F32, tag="zero")
            nc.gpsimd.memset(zt[:], 0.0)
            for i in range(reps * (ncols // tile_f)):
                i = i % (ncols // tile_f)
                sl = slice(i * tile_f, (i + 1) * tile_f)
                xt = pool.tile([P, tile_f], _F32, tag="x")
                lt = pool.tile([P, tile_f], _F32, tag="l")
                ut = pool.tile([P, tile_f], _F32, tag="u")
                nc.sync.dma_start(out=xt[:], in_=x[:, sl])
                nc.sync.dma_start(out=lt[:], in_=lo[:, sl])
                nc.sync.dma_start(out=ut[:], in_=up[:, sl])

                nc.scalar.activation(xt[:], xt[:], _RELU)  # x_out, in place
                nc.scalar.activation(ut[:], ut[:], _RELU)  # up = relu(u)
                nlt = pool.tile([P, tile_f], _F32, tag="nl")
                nc.scalar.activation(nlt[:], lt[:], _RELU, scale=-1.0)  # relu(-l)
                sqt = pool.tile([P, tile_f], _F32, tag="sq")
                nc.scalar.activation(sqt[:], ut[:], _SQUARE)  # up^2

                # exact masks; HW CopyPredicated requires an integer mask
                # dtype.  is_ge (not Relu(l)!) so l == 0.0 takes the pos
                # branch exactly like the reference; is_le on relu(u) is
                # exactly (upper <= 0), -0.0 included.
                ppt = pool.tile([P, tile_f], _U8, tag="pp")
                nc.vector.tensor_scalar(
                    out=ppt[:], in0=lt[:], scalar1=0.0, scalar2=None,
                    op0=mybir.AluOpType.is_ge,
                )
                let = pool.tile([P, tile_f], _U8, tag="le")
                nc.gpsimd.tensor_scalar(
                    out=let[:], in0=ut[:], scalar1=0.0, scalar2=None,
                    op0=mybir.AluOpType.is_le,
                )

                nc.gpsimd.tensor_add(out=nlt[:], in0=ut[:], in1=nlt[:])  # d
                nc.vector.reciprocal_approx_fast(out=nlt[:], in_=nlt[:])  # r
                nc.vector.tensor_mul(out=sqt[:], in0=sqt[:], in1=nlt[:])  # uo

                nc.vector.copy_predicated(out=lt[:], mask=let[:], data=zt[:])
                nc.vector.copy_predicated(out=lt[:], mask=ppt[:], data=sqt[:])

                nc.sync.dma_start(out=xo[:, sl], in_=xt[:])
                nc.sync.dma_start(out=loo[:, sl], in_=lt[:])
                nc.sync.dma_start(out=upo[:, sl], in_=sqt[:])
    nc.compile()
    return nc


def run(inputs: dict, trace: bool = False):
    """Shard, execute on 8 cores, gather. Returns (outputs_tuple, results_obj)."""
    arrs = {}
    for k in ("x", "lower", "upper"):
        a = np.asarray(inputs[k], dtype=np.float32)
        arrs[k] = np.ascontiguousarray(a).reshape(N_CORES, P, NCOLS)
    in_maps = [
        {k: arrs[k][c] for k in ("x", "lower", "upper")} for c in range(N_CORES)
    ]
    nc = build_nc()
    res = bass_utils.run_bass_kernel_spmd(
        nc, in_maps, core_ids=list(range(N_CORES)), trace=trace
    )
    outs = []
    for name in ("x_out", "lower_out", "upper_out"):
        full = np.stack([res.results[c][name] for c in range(N_CORES)])
        outs.append(full.reshape(1, N_TOTAL).astype(np.float32, copy=False))
    return tuple(outs), res


def kernel(**inputs):
    outs, _ = run(inputs, trace=False)
    return outs
